# revision 7
# baseline (speedup 1.0000x reference)
"""MoE layer (8 experts, top-2 sigmoid routing, SwiGLU experts + shared expert)
on 8 TRN2 NeuronCores.

Strategy (expert-parallel, host-side token dispatch):
  - Router (sigmoid(x @ gate_w.T), top-2, weight normalization) is tiny
    (~50 MFLOP) and runs on the host; it determines the all-to-all dispatch.
  - Core c owns expert c: it gets the tokens routed to expert c (gathered and
    zero-padded to a common capacity M_pad) plus expert c's Wi/Wo.
  - The shared expert is data-parallel: core c also processes tokens
    [c*512, (c+1)*512) with the (replicated) shared weights.
  - Device kernel computes the two SwiGLU MLP passes in bf16 (fp32 PSUM
    accumulation), feature-major layout (features on partitions, tokens on the
    free dim) so no on-device transposes are needed.
  - Host combine: out[t] = shared_out[t] + sum_e cw[e,t] * expert_out[e][t]
    (the combine weights are applied on the host during the scatter-add).
"""

from contextlib import ExitStack

import ml_dtypes
import numpy as np

import concourse.bass as bass
import concourse.tile as tile
from concourse import bacc, mybir
from concourse.bass_utils import run_bass_kernel_spmd

E, TOPK, H, I = 8, 2, 768, 1152
I2 = 2 * I
T = 4096
N_CORES = 8
TS = T // N_CORES  # shared-expert tokens per core
P = 128
KH = H // P    # 6 contraction tiles over H
KI = I // P    # 9 contraction tiles over I
BF16 = mybir.dt.bfloat16
F32 = mybir.dt.float32
MAXN = 512     # max tokens per matmul chunk (one fp32 PSUM bank)

_BUILD_CACHE: dict = {}
LAST_RESULTS = None  # BassKernelResults of the most recent device run


def _ensure_axon_ntff_hook():
    """This image's `antenv` lacks the `axon_hooks` module that
    run_bass_kernel_spmd imports when NTFF tracing is requested (BASS_TRACE=1).
    Install an equivalent shim so profiling works instead of crashing."""
    try:
        import antenv.axon_hooks  # noqa: F401
        return
    except ImportError:
        pass
    import sys
    import types
    try:
        import antenv
    except ImportError:
        return
    mod = types.ModuleType("antenv.axon_hooks")
    holder = {"hook": None}
    mod.set_axon_ntff_profile_hook = lambda h: holder.__setitem__("hook", h)
    mod.get_axon_ntff_profile_hook = lambda: holder["hook"]
    sys.modules["antenv.axon_hooks"] = mod
    antenv.axon_hooks = mod
    so_path = "/opt/axon/libaxon_pjrt.so"
    try:
        import os
        if os.path.exists(so_path):
            from trn_agent_boot.trn_boot import _ntff_profile_via_ctypes
            hook = _ntff_profile_via_ctypes(so_path)
            if hook is not None:
                mod.set_axon_ntff_profile_hook(hook)
    except Exception:
        pass  # hook stays None; bass_utils logs a warning and skips tracing


def _chunk_sizes(m: int) -> list[int]:
    """Split m (multiple of 128) into ceil(m/512) near-equal multiples of 128."""
    q = m // P
    n = -(-m // MAXN)
    return [P * (q // n + (1 if i < q % n else 0)) for i in range(n)]


def _build(m_pad: int):
    nc = bacc.Bacc("TRN2", target_bir_lowering=False, debug=False,
                   num_devices=N_CORES)

    xe = nc.dram_tensor("xe", [H, m_pad], BF16, kind="ExternalInput").ap()
    wi = nc.dram_tensor("wi", [H, I2], BF16, kind="ExternalInput").ap()
    wo = nc.dram_tensor("wo", [I, H], BF16, kind="ExternalInput").ap()
    xs = nc.dram_tensor("xs", [H, TS], BF16, kind="ExternalInput").ap()
    swi = nc.dram_tensor("swi", [H, I2], BF16, kind="ExternalInput").ap()
    swo = nc.dram_tensor("swo", [I, H], BF16, kind="ExternalInput").ap()
    ye = nc.dram_tensor("ye", [H, m_pad], F32, kind="ExternalOutput").ap()
    ys = nc.dram_tensor("ys", [H, TS], F32, kind="ExternalOutput").ap()

    with ExitStack() as ctx:
        tc = ctx.enter_context(tile.TileContext(nc))
        wpool = ctx.enter_context(tc.tile_pool(name="weights", bufs=1))
        xpool = ctx.enter_context(tc.tile_pool(name="x", bufs=3))
        apool = ctx.enter_context(tc.tile_pool(name="act", bufs=3))
        spool = ctx.enter_context(tc.tile_pool(name="silu", bufs=3))
        ypool = ctx.enter_context(tc.tile_pool(name="y", bufs=3))
        psum = ctx.enter_context(tc.tile_pool(name="psum", bufs=6, space="PSUM"))

        def load_w(dram_ap, ktiles, tag):
            t = wpool.tile([P, ktiles, dram_ap.shape[1]], BF16, tag=tag, name=tag)
            nc.sync.dma_start(t[:], dram_ap.rearrange("(o p) f -> p o f", p=P))
            return t

        wi_sb = load_w(wi, KH, "wi")
        wo_sb = load_w(wo, KI, "wo")
        swi_sb = load_w(swi, KH, "swi")
        swo_sb = load_w(swo, KI, "swo")

        # (x_dram, y_dram, wi_sbuf, wo_sbuf, chunk_off, chunk_sz, silu_on_first)
        chunks = []
        for xd, yd, wis, wos, m, sfirst in (
            (xe, ye, wi_sb, wo_sb, m_pad, False),  # experts: silu(gate=2nd half)*proj
            (xs, ys, swi_sb, swo_sb, TS, True),    # shared: silu(s_in=1st half)*s_gate
        ):
            off = 0
            for sz in _chunk_sizes(m):
                chunks.append((xd, yd, wis, wos, off, sz, sfirst))
                off += sz

        def emit_wi(c):
            xd, yd, wis, wos, off, sz, sfirst = chunks[c]
            xt = xpool.tile([P, KH, MAXN], BF16, tag="xt", name="xt")[:, :, :sz]
            nc.sync.dma_start(
                xt, xd.rearrange("(o p) m -> p o m", p=P)[:, :, off:off + sz])
            act = apool.tile([P, KI, MAXN], BF16, tag="act", name="act")[:, :, :sz]
            for ft in range(KI):
                ps_a = psum.tile([P, MAXN], F32, tag="ps", name="ps_a")[:, :sz]  # 1st-half tile
                for kt in range(KH):
                    nc.tensor.matmul(ps_a, lhsT=wis[:, kt, ft * P:(ft + 1) * P],
                                     rhs=xt[:, kt, :],
                                     start=(kt == 0), stop=(kt == KH - 1))
                ps_b = psum.tile([P, MAXN], F32, tag="ps", name="ps_b")[:, :sz]  # 2nd-half tile
                for kt in range(KH):
                    nc.tensor.matmul(ps_b,
                                     lhsT=wis[:, kt, (KI + ft) * P:(KI + ft + 1) * P],
                                     rhs=xt[:, kt, :],
                                     start=(kt == 0), stop=(kt == KH - 1))
                sl = spool.tile([P, MAXN], F32, tag="silu", name="sl")[:, :sz]
                tmp = spool.tile([P, MAXN], F32, tag="silu2", name="tmp")[:, :sz]
                ps_s, ps_m = (ps_a, ps_b) if sfirst else (ps_b, ps_a)
                # silu(s) * m, with silu(s) = s * sigmoid(s)
                nc.scalar.activation(sl, ps_s, mybir.ActivationFunctionType.Sigmoid)
                nc.vector.tensor_mul(tmp, sl, ps_s)
                nc.vector.tensor_mul(act[:, ft, :], tmp, ps_m)
            return act

        def emit_wo(c, act):
            xd, yd, wis, wos, off, sz, sfirst = chunks[c]
            for ht in range(KH):
                ps_y = psum.tile([P, MAXN], F32, tag="ps", name="ps_y")[:, :sz]
                for kt in range(KI):
                    nc.tensor.matmul(ps_y, lhsT=wos[:, kt, ht * P:(ht + 1) * P],
                                     rhs=act[:, kt, :],
                                     start=(kt == 0), stop=(kt == KI - 1))
                yt = ypool.tile([P, MAXN], F32, tag="y", name="yt")[:, :sz]
                nc.vector.tensor_copy(yt, ps_y)
                nc.sync.dma_start(
                    yd.rearrange("(o p) m -> p o m", p=P)[:, ht, off:off + sz], yt)

        # software pipeline: Wi(c+1) is emitted before Wo(c) so the PE always
        # has independent matmul work while ACT/DVE finish chunk c's SwiGLU
        acts = [None] * len(chunks)
        acts[0] = emit_wi(0)
        for c in range(1, len(chunks)):
            acts[c] = emit_wi(c)
            emit_wo(c - 1, acts[c - 1])
        emit_wo(len(chunks) - 1, acts[-1])

    nc.compile()
    return nc


def _route(x, gate_w, correction_bias):
    logits = 1.0 / (1.0 + np.exp(-(x @ gate_w.T), dtype=np.float32))  # (T, E)
    sel = logits + correction_bias[None, :]
    order = np.argsort(-sel, axis=1, kind="stable")[:, :TOPK]  # ties -> low index
    w = np.take_along_axis(logits, order, axis=1)
    w = (w / w.sum(axis=1, keepdims=True)).astype(np.float32)
    return order, w


def kernel(**inputs) -> np.ndarray:
    x = np.asarray(inputs["x"], np.float32)
    gate_w = np.asarray(inputs["gate_w"], np.float32)
    bias = np.asarray(inputs["correction_bias"], np.float32)
    Wi = np.asarray(inputs["Wi"], np.float32)
    Wo = np.asarray(inputs["Wo"], np.float32)
    shared_Wi = np.asarray(inputs["shared_Wi"], np.float32)
    shared_Wo = np.asarray(inputs["shared_Wo"], np.float32)

    order, w = _route(x, gate_w, bias)

    idx_per_e, cw_per_e = [], []
    for e in range(E):
        mask = order == e  # (T, K)
        tok = mask.any(axis=1)
        rows = np.nonzero(tok)[0]
        kpos = np.argmax(mask[rows], axis=1)
        idx_per_e.append(rows)
        cw_per_e.append(w[rows, kpos].astype(np.float32))

    m_pad = max(P, -(-max(len(r) for r in idx_per_e) // P) * P)

    bf = ml_dtypes.bfloat16
    xT = np.ascontiguousarray(x.T)  # (H, T) f32
    swiT = np.ascontiguousarray(shared_Wi.T).astype(bf)  # (H, 2I)
    swoT = np.ascontiguousarray(shared_Wo.T).astype(bf)  # (I, H)

    in_maps = []
    for c in range(N_CORES):
        rows = idx_per_e[c]
        xe = np.zeros((H, m_pad), bf)
        xe[:, :len(rows)] = xT[:, rows].astype(bf)
        in_maps.append({
            "xe": xe,
            "wi": Wi[c].astype(bf),                      # (H, 2I)
            "wo": Wo[c].astype(bf),                      # (I, H)
            "xs": np.ascontiguousarray(
                xT[:, c * TS:(c + 1) * TS]).astype(bf),  # (H, TS)
            "swi": swiT,
            "swo": swoT,
        })

    if m_pad not in _BUILD_CACHE:
        _BUILD_CACHE[m_pad] = _build(m_pad)
    nc = _BUILD_CACHE[m_pad]

    _ensure_axon_ntff_hook()
    res = run_bass_kernel_spmd(nc, in_maps, list(range(N_CORES)))
    global LAST_RESULTS
    LAST_RESULTS = res

    out = np.zeros((T, H), np.float32)
    for c in range(N_CORES):
        r = res.results[c]
        out[c * TS:(c + 1) * TS] += r["ys"].T
        rows = idx_per_e[c]
        if len(rows):
            out[rows] += r["ye"][:, :len(rows)].T * cw_per_e[c][:, None]
    return out


# revision 9
# speedup vs baseline: 1.2045x; 1.2045x over previous
"""MoE layer (8 experts, top-2 sigmoid routing, SwiGLU experts + shared expert)
on 8 TRN2 NeuronCores.

Strategy (expert-parallel, host-side token dispatch):
  - Router (sigmoid(x @ gate_w.T), top-2, weight normalization) is tiny
    (~50 MFLOP) and runs on the host; it determines the all-to-all dispatch.
  - Core c owns expert c: it gets the tokens routed to expert c (gathered and
    zero-padded to a common capacity M_pad) plus expert c's Wi/Wo.
  - The shared expert is data-parallel: core c also processes tokens
    [c*512, (c+1)*512) with the (replicated) shared weights.
  - Device kernel computes the two SwiGLU MLP passes in bf16 (fp32 PSUM
    accumulation), feature-major layout (features on partitions, tokens on the
    free dim) so no on-device transposes are needed.
  - Host combine: out[t] = shared_out[t] + sum_e cw[e,t] * expert_out[e][t]
    (the combine weights are applied on the host during the scatter-add).
"""

from contextlib import ExitStack

import ml_dtypes
import numpy as np

import concourse.bass as bass
import concourse.tile as tile
from concourse import bacc, mybir
from concourse.bass_utils import run_bass_kernel_spmd

E, TOPK, H, I = 8, 2, 768, 1152
I2 = 2 * I
T = 4096
N_CORES = 8
TS = T // N_CORES  # shared-expert tokens per core
P = 128
KH = H // P    # 6 contraction tiles over H
KI = I // P    # 9 contraction tiles over I
BF16 = mybir.dt.bfloat16
F32 = mybir.dt.float32
MAXN = 512     # max tokens per matmul chunk (one fp32 PSUM bank)

_BUILD_CACHE: dict = {}
LAST_RESULTS = None  # BassKernelResults of the most recent device run


def _ensure_axon_ntff_hook():
    """This image's `antenv` lacks the `axon_hooks` module that
    run_bass_kernel_spmd imports when NTFF tracing is requested (BASS_TRACE=1).
    Install an equivalent shim so profiling works instead of crashing."""
    try:
        import antenv.axon_hooks  # noqa: F401
        return
    except ImportError:
        pass
    import sys
    import types
    try:
        import antenv
    except ImportError:
        return
    mod = types.ModuleType("antenv.axon_hooks")
    holder = {"hook": None}
    mod.set_axon_ntff_profile_hook = lambda h: holder.__setitem__("hook", h)
    mod.get_axon_ntff_profile_hook = lambda: holder["hook"]
    sys.modules["antenv.axon_hooks"] = mod
    antenv.axon_hooks = mod
    so_path = "/opt/axon/libaxon_pjrt.so"
    try:
        import os
        if os.path.exists(so_path):
            from trn_agent_boot.trn_boot import _ntff_profile_via_ctypes
            hook = _ntff_profile_via_ctypes(so_path)
            if hook is not None:
                mod.set_axon_ntff_profile_hook(hook)
    except Exception:
        pass  # hook stays None; bass_utils logs a warning and skips tracing


def _chunk_sizes(m: int) -> list[int]:
    """Split m (multiple of 128) into ceil(m/512) near-equal multiples of 128."""
    q = m // P
    n = -(-m // MAXN)
    return [P * (q // n + (1 if i < q % n else 0)) for i in range(n)]


def _build(m_pad: int):
    nc = bacc.Bacc("TRN2", target_bir_lowering=False, debug=False,
                   num_devices=N_CORES)

    xe = nc.dram_tensor("xe", [H, m_pad], BF16, kind="ExternalInput").ap()
    wi = nc.dram_tensor("wi", [H, I2], BF16, kind="ExternalInput").ap()
    wo = nc.dram_tensor("wo", [I, H], BF16, kind="ExternalInput").ap()
    xs = nc.dram_tensor("xs", [H, TS], BF16, kind="ExternalInput").ap()
    swi = nc.dram_tensor("swi", [H, I2], BF16, kind="ExternalInput").ap()
    swo = nc.dram_tensor("swo", [I, H], BF16, kind="ExternalInput").ap()
    ye = nc.dram_tensor("ye", [H, m_pad], F32, kind="ExternalOutput").ap()
    ys = nc.dram_tensor("ys", [H, TS], F32, kind="ExternalOutput").ap()

    with ExitStack() as ctx:
        tc = ctx.enter_context(tile.TileContext(nc))
        wpool = ctx.enter_context(tc.tile_pool(name="weights", bufs=1))
        apool = ctx.enter_context(tc.tile_pool(name="act", bufs=3))
        spool = ctx.enter_context(tc.tile_pool(name="silu", bufs=3))
        ypool = ctx.enter_context(tc.tile_pool(name="y", bufs=3))
        psum = ctx.enter_context(tc.tile_pool(name="psum", bufs=6, space="PSUM"))

        # All resident tensors (x and weights) are loaded as per-k-tile SBUF
        # tiles: dependency tracking is per tile, so a matmul only waits for
        # the one 128-row slice it reads, and compute starts as soon as the
        # first slices land instead of after the whole 13 MB preload.
        def load_rows(dram_ap, ktiles, tag):
            src = dram_ap.rearrange("(o p) f -> p o f", p=P)
            out = []
            for kt in range(ktiles):
                t = wpool.tile([P, dram_ap.shape[1]], BF16, tag=f"{tag}{kt}",
                               name=f"{tag}{kt}")
                nc.sync.dma_start(t[:], src[:, kt])
                out.append(t)
            return out

        xe_t = load_rows(xe, KH, "xe")      # tokens first: small, needed first
        wi_t = load_rows(wi, KH, "wi")

        # (x_tiles, y_dram, wi_tiles, wo_tiles, chunk_off, chunk_sz, silu_first)
        chunks = []
        for xts, yd, wis, wos, m, sfirst in (
            (xe_t, ye, wi_t, "wo", m_pad, False),  # experts: silu(2nd half)*1st
            ("xs", ys, "swi", "swo", TS, True),    # shared: silu(1st half)*2nd
        ):
            off = 0
            for sz in _chunk_sizes(m):
                chunks.append([xts, yd, wis, wos, off, sz, sfirst])
                off += sz

        def emit_wi(c):
            xts, yd, wis, wos, off, sz, sfirst = chunks[c]
            act = apool.tile([P, KI, MAXN], BF16, tag="act", name="act")[:, :, :sz]
            for ft in range(KI):
                ps_a = psum.tile([P, MAXN], F32, tag="ps", name="ps_a")[:, :sz]
                for kt in range(KH):
                    nc.tensor.matmul(ps_a, lhsT=wis[kt][:, ft * P:(ft + 1) * P],
                                     rhs=xts[kt][:, off:off + sz],
                                     start=(kt == 0), stop=(kt == KH - 1))
                ps_b = psum.tile([P, MAXN], F32, tag="ps", name="ps_b")[:, :sz]
                for kt in range(KH):
                    nc.tensor.matmul(ps_b,
                                     lhsT=wis[kt][:, (KI + ft) * P:(KI + ft + 1) * P],
                                     rhs=xts[kt][:, off:off + sz],
                                     start=(kt == 0), stop=(kt == KH - 1))
                sl = spool.tile([P, MAXN], F32, tag="silu", name="sl")[:, :sz]
                tmp = spool.tile([P, MAXN], F32, tag="silu2", name="tmp")[:, :sz]
                ps_s, ps_m = (ps_a, ps_b) if sfirst else (ps_b, ps_a)
                # silu(s) * m, with silu(s) = s * sigmoid(s)
                nc.scalar.activation(sl, ps_s, mybir.ActivationFunctionType.Sigmoid)
                nc.vector.tensor_mul(tmp, sl, ps_s)
                nc.vector.tensor_mul(act[:, ft, :], tmp, ps_m)
            return act

        def emit_wo(c, act):
            xts, yd, wis, wos, off, sz, sfirst = chunks[c]
            for ht in range(KH):
                ps_y = psum.tile([P, MAXN], F32, tag="ps", name="ps_y")[:, :sz]
                for kt in range(KI):
                    nc.tensor.matmul(ps_y, lhsT=wos[kt][:, ht * P:(ht + 1) * P],
                                     rhs=act[:, kt, :],
                                     start=(kt == 0), stop=(kt == KI - 1))
                yt = ypool.tile([P, MAXN], F32, tag="y", name="yt")[:, :sz]
                nc.vector.tensor_copy(yt, ps_y)
                nc.sync.dma_start(
                    yd.rearrange("(o p) m -> p o m", p=P)[:, ht, off:off + sz], yt)

        # software pipeline: Wi(c+1) is emitted before Wo(c) so the PE always
        # has independent matmul work while ACT/DVE finish chunk c's SwiGLU.
        # Remaining weight/x loads are emitted at the latest point that still
        # leaves a full compute chunk of DMA lead time.
        n = len(chunks)
        acts = [None] * n
        acts[0] = emit_wi(0)
        wo_t = load_rows(wo, KI, "wo")
        xs_t = load_rows(xs, KH, "xs")
        swi_t = load_rows(swi, KH, "swi")
        swo_t = load_rows(swo, KI, "swo")
        named = {"xs": xs_t, "swi": swi_t, "wo": wo_t, "swo": swo_t}
        for ch in chunks:
            for j in (0, 2, 3):
                if isinstance(ch[j], str):
                    ch[j] = named[ch[j]]
        for c in range(1, n):
            acts[c] = emit_wi(c)
            emit_wo(c - 1, acts[c - 1])
        emit_wo(n - 1, acts[-1])

    nc.compile()
    return nc


def _route(x, gate_w, correction_bias):
    logits = 1.0 / (1.0 + np.exp(-(x @ gate_w.T), dtype=np.float32))  # (T, E)
    sel = logits + correction_bias[None, :]
    order = np.argsort(-sel, axis=1, kind="stable")[:, :TOPK]  # ties -> low index
    w = np.take_along_axis(logits, order, axis=1)
    w = (w / w.sum(axis=1, keepdims=True)).astype(np.float32)
    return order, w


def kernel(**inputs) -> np.ndarray:
    x = np.asarray(inputs["x"], np.float32)
    gate_w = np.asarray(inputs["gate_w"], np.float32)
    bias = np.asarray(inputs["correction_bias"], np.float32)
    Wi = np.asarray(inputs["Wi"], np.float32)
    Wo = np.asarray(inputs["Wo"], np.float32)
    shared_Wi = np.asarray(inputs["shared_Wi"], np.float32)
    shared_Wo = np.asarray(inputs["shared_Wo"], np.float32)

    order, w = _route(x, gate_w, bias)

    idx_per_e, cw_per_e = [], []
    for e in range(E):
        mask = order == e  # (T, K)
        tok = mask.any(axis=1)
        rows = np.nonzero(tok)[0]
        kpos = np.argmax(mask[rows], axis=1)
        idx_per_e.append(rows)
        cw_per_e.append(w[rows, kpos].astype(np.float32))

    m_pad = max(P, -(-max(len(r) for r in idx_per_e) // P) * P)

    bf = ml_dtypes.bfloat16
    xT = np.ascontiguousarray(x.T)  # (H, T) f32
    swiT = np.ascontiguousarray(shared_Wi.T).astype(bf)  # (H, 2I)
    swoT = np.ascontiguousarray(shared_Wo.T).astype(bf)  # (I, H)

    in_maps = []
    for c in range(N_CORES):
        rows = idx_per_e[c]
        xe = np.zeros((H, m_pad), bf)
        xe[:, :len(rows)] = xT[:, rows].astype(bf)
        in_maps.append({
            "xe": xe,
            "wi": Wi[c].astype(bf),                      # (H, 2I)
            "wo": Wo[c].astype(bf),                      # (I, H)
            "xs": np.ascontiguousarray(
                xT[:, c * TS:(c + 1) * TS]).astype(bf),  # (H, TS)
            "swi": swiT,
            "swo": swoT,
        })

    if m_pad not in _BUILD_CACHE:
        _BUILD_CACHE[m_pad] = _build(m_pad)
    nc = _BUILD_CACHE[m_pad]

    _ensure_axon_ntff_hook()
    res = run_bass_kernel_spmd(nc, in_maps, list(range(N_CORES)))
    global LAST_RESULTS
    LAST_RESULTS = res

    out = np.zeros((T, H), np.float32)
    for c in range(N_CORES):
        r = res.results[c]
        out[c * TS:(c + 1) * TS] += r["ys"].T
        rows = idx_per_e[c]
        if len(rows):
            out[rows] += r["ye"][:, :len(rows)].T * cw_per_e[c][:, None]
    return out
